# revision 5
# baseline (speedup 1.0000x reference)
"""Trainium2 Bass kernel for the AI4Burgers 3x3-stencil operator.

Reference computation (per batch image, replicate padding):
    Lu = NU*conv3x3(u, w1) - u_vel*conv3x3(u, w2) - u_vel*conv3x3(u, w3)
       = conv3x3(u, NU*w1) - u_vel * conv3x3(u, w2 + w3)

Strategy
- Data-parallel over batch: 16 images across 8 NeuronCores, 2 images/core.
  Each core's 2 images form one flat [2048, 1024] sheet cut into 17 row
  chunks (H on the SBUF partition axis, W on the free axis). The vertical
  stencil is a banded [K, M] stationary matrix on the TensorEngine; the
  horizontal taps come from 3 PSUM-accumulated matmuls on column-shifted
  views of the same SBUF tile.
- Only 2 band types exist: interior and the img0|img1 straddle chunk
  (clamping folded into 2 special band columns). Image top/bottom replicate
  padding is realized by duplicating one input row in the DMA, so the top
  and bottom chunks reuse the interior bands. Stat = 12 segments (393KB)
  instead of 24 (786KB), split into 3 priority-ordered DMA triggers so the
  first matmul is gated by only 98KB on a low-latency HWDGE ring.
- First chunk is small (M=34) so its u load (74KB) lands early; it avoids
  edge-column fills via N=1 edge matmuls. A few dummy matmuls on a zeroed
  tile warm the PE p-state ramp during the DMA fill.
- u rides SWDGE (16-way SDMA spread); u_vel loads and output stores ride
  the two HWDGE rings, pair-batched (2 chunks per descriptor) to halve
  trigger-op and semaphore pressure. The pointwise combine runs on the DVE
  reading PSUM directly (no ACT evacuation); the last chunk combines and
  stores per-512-column half to shorten the tail.
"""

import numpy as np

NU = 0.5
B, H, W = 16, 1024, 1024
NCORES = 8
IMGS_PER_CORE = B // NCORES
FH = H * IMGS_PER_CORE  # flat sheet rows per core

# chunk table: (c, K, M, rin, rout, ctype)
# ctype: 0=top (dup row0 -> partition 0), 1=interior, 2=straddle,
#        3=bottom (dup last row -> partition K-1)
CHUNKS = []
CHUNKS.append((0, 36, 34, 0, 0, 0))
_r = 34
for _ in range(7):
    CHUNKS.append((len(CHUNKS), 128, 126, _r - 1, _r, 1))
    _r += 126
STRADDLE_ROUT = _r  # 916
CHUNKS.append((len(CHUNKS), 126, 124, _r - 1, _r, 2))
_r += 124
for _i in range(8):
    ct = 3 if _i == 7 else 1
    CHUNKS.append((len(CHUNKS), 128, 126, _r - 1, _r, ct))
    _r += 126
assert _r == FH
NCHUNK = len(CHUNKS)  # 17

# stat segments: [conv1-int dx0..2 | conv0-int dx0..2 | conv1-str | conv0-str]
NSEG = 12

# v/out pair batching: chunks grouped per DMA descriptor
PAIRS = [(0,), (1, 2), (3, 4), (5, 6), (7,), (8,), (9, 10), (11, 12),
         (13, 14), (15,), (16,)]

_cache = {}


def _interior_seg(w3):
    """[128,128] band S[k,m] = w3[k-m] for k-m in 0..2 (k<=127)."""
    S = np.zeros((128, 128), np.float64)
    for d in range(3):
        for m in range(128 - d):
            S[m + d, m] = w3[d]
    return S


def _straddle_seg(w3):
    S = _interior_seg(w3)
    mb = (H - 1) - STRADDLE_ROUT  # 107: output m of flat row 1023
    S[:, mb] = 0
    S[mb, mb] = w3[0]
    S[mb + 1, mb] = w3[1] + w3[2]
    S[:, mb + 1] = 0
    S[mb + 2, mb + 1] = w3[0] + w3[1]
    S[mb + 3, mb + 1] = w3[2]
    return S


def _build_stationaries(w1, w2, w3):
    wa = NU * np.asarray(w1, np.float64)[0, 0]
    wb = (np.asarray(w2, np.float64) + np.asarray(w3, np.float64))[0, 0]
    stat = np.zeros((128, NSEG * 128), np.float64)
    for base, wm, builder in (
        (0, wb, _interior_seg),
        (3, wa, _interior_seg),
        (6, wb, _straddle_seg),
        (9, wa, _straddle_seg),
    ):
        for dx in range(3):
            s = base + dx
            stat[:, s * 128 : (s + 1) * 128] = builder(wm[:, dx])
    return stat.astype(np.float32)


def _build_program():
    from concourse import bacc, tile, mybir
    from concourse.tile import add_dep_helper

    f32 = mybir.dt.float32
    bf16 = mybir.dt.bfloat16

    nc = bacc.Bacc(None, target_bir_lowering=False, debug=False)
    u_d = nc.dram_tensor("u", [FH, W], bf16, kind="ExternalInput").ap()
    v_d = nc.dram_tensor("uvel", [FH, W], bf16, kind="ExternalInput").ap()
    s_d = nc.dram_tensor("stat", [128, NSEG * 128], bf16, kind="ExternalInput").ap()
    o_d = nc.dram_tensor("out", [FH, W], bf16, kind="ExternalOutput").ap()

    # u tile: data cols [GO, GO+W). GO=16 bf16 elems = 32B so DMA writes land
    # 32B-aligned; width 1056 keeps the partition pitch (2112B) a multiple of
    # 32B and off the power-of-2 stride that causes SBUF bank conflicts.
    GO = 16
    UTW = 1056

    # chunk -> (pair_index, half, pair) lookup
    chunk_pair = {}
    for pi, pr in enumerate(PAIRS):
        for half, cc in enumerate(pr):
            chunk_pair[cc] = (pi, half, pr)

    with tile.TileContext(nc) as tc:
        with (
            tc.tile_pool(name="const", bufs=1) as cp,
            tc.tile_pool(name="up", bufs=6) as up,
            tc.tile_pool(name="vp", bufs=4) as vp,
            tc.tile_pool(name="op", bufs=4) as op,
            tc.tile_pool(name="tp", bufs=6) as tp,
            tc.tile_pool(name="pp", bufs=2, space="PSUM") as pp,
        ):
            stat_t = cp.tile([128, NSEG * 128], bf16)
            # priority-ordered stat loads: conv1-interior gates the first
            # matmul (98KB on the low-latency sync HWDGE ring), conv0-interior
            # right behind it; the straddle segs aren't read until chunk 8 so
            # they ride the scalar ring after chunk 0's u.
            # the whole pipeline-start chain (conv1-interior stat + chunk
            # 0's u) rides ONE HWDGE ring so SWDGE burst contention cannot
            # starve it; conv0-interior + straddle stats ride the other ring.
            nc.sync.dma_start(stat_t[:, 0:384], s_d[:, 0:384])
            ut0 = up.tile([128, UTW], bf16, tag="ut")
            nc.sync.dma_start(ut0[0:1, GO : GO + W], u_d[0:1, :])
            ui0 = nc.sync.dma_start(ut0[1:36, GO : GO + W], u_d[0:35, :])
            nc.scalar.dma_start(stat_t[:, 384:768], s_d[:, 384:768])
            nc.scalar.dma_start(stat_t[:, 768:1536], s_d[:, 768:1536])

            # PE p-state warm-up: the PE ramps to full clock only after ~3us
            # of continuous execution. Burn dummy matmuls on a zeroed tile
            # during the DMA fill so the real stream starts near full speed.
            dmy = cp.tile([128, 640], bf16, tag="dmy")
            nc.vector.memset(dmy[0:128, 0:128], 0.0)
            nc.vector.memset(dmy[0:128, 128:640], 0.0)
            pD = pp.tile([128, 1024], f32, tag="pA", name="pD")
            for i in range(5):
                nc.tensor.matmul(
                    pD[0:128, 0:512],
                    dmy[0:128, 0:128],
                    dmy[0:128, 128:640],
                    start=(i == 0),
                    stop=(i == 4),
                )

            pair_state = {}  # pair_index -> (vt2, ot2)

            for c, K, M, rin, rout, ct in CHUNKS:
                pi, half, pr = chunk_pair[c]
                veng = nc.sync if pi % 2 == 0 else nc.scalar
                oeng = nc.scalar if pi % 2 == 0 else nc.sync

                # ---- u load (chunk 0 already issued above) ----
                if ct == 0:
                    ut, ui = ut0, ui0
                else:
                    ut = up.tile([128, UTW], bf16, tag="ut")
                    if ct == 3:
                        # bottom: rows rin..2047 -> p0..126, dup 2047 -> p127
                        nc.gpsimd.dma_start(
                            ut[K - 1 : K, GO : GO + W], u_d[FH - 1 : FH, :]
                        )
                        ui = nc.gpsimd.dma_start(
                            ut[0 : K - 1, GO : GO + W], u_d[rin : rin + K - 1, :]
                        )
                    else:
                        ui = nc.gpsimd.dma_start(
                            ut[0:K, GO : GO + W], u_d[rin : rin + K, :]
                        )
                        if c == 1:
                            # keep the SWDGE burst from contending with the
                            # fill-critical c0 load on the HWDGE ring
                            add_dep_helper(
                                ui.ins, ui0.ins, sync=True, reason="u1 after u0"
                            )
                    # W-edge replicate columns filled on ACT: prefetch depth
                    # means the u-DMA sem is long satisfied when ACT gets here
                    nc.scalar.copy(ut[0:K, GO - 1 : GO], ut[0:K, GO : GO + 1])
                    nc.scalar.copy(
                        ut[0:K, GO + W : GO + W + 1],
                        ut[0:K, GO + W - 1 : GO + W],
                    )

                # ---- u_vel load: one DMA per pair, at the pair's 1st chunk ----
                if half == 0:
                    vt2 = vp.tile([128, 2048], bf16, tag="vt2")
                    ot2 = op.tile([128, 2048], bf16, tag="ot2")
                    pair_state[pi] = (vt2, ot2)
                    if len(pr) == 2:
                        # SBUF APs must be partition-major: [p, s, w]
                        src = v_d[rout : rout + 2 * M, :].rearrange(
                            "(s p) w -> p s w", s=2
                        )
                        dst = vt2[0:M, :].rearrange("p (s w) -> p s w", s=2)
                        vi = veng.dma_start(dst, src)
                    else:
                        vi = veng.dma_start(vt2[0:M, 0:W], v_d[rout : rout + M, :])
                    # u_vel load waits for this chunk's u load: the HWDGE
                    # burst must not crowd HBM while u (the critical path)
                    # is draining
                    add_dep_helper(vi.ins, ui.ins, sync=True, reason="vt after ut")
                vt2, ot2 = pair_state[pi]
                vcol = half * 1024

                # ---- matmuls ----
                pA = pp.tile([128, 1024], f32, tag="pA", name="pA")
                pB = pp.tile([128, 1024], f32, tag="pB", name="pB")
                tt = tp.tile([128, W], bf16, tag="tt")
                segoff = 6 if ct == 2 else 0
                # conv1 (pB) runs first so the DVE multiply overlaps conv0's
                # matmuls instead of waiting for all of them.
                for conv, pt in ((1, pB), (0, pA)):
                    segb = ((0 if conv == 1 else 3) + segoff) * 128
                    s0 = stat_t[0:K, segb : segb + 128]
                    s1 = stat_t[0:K, segb + 128 : segb + 256]
                    s2 = stat_t[0:K, segb + 256 : segb + 384]
                    if ct == 0:
                        # first chunk avoids the ACT edge fills entirely
                        # (N=1 edge matmuls instead) so nothing gates the
                        # pipeline start
                        nc.tensor.matmul(
                            pt[0:128, 1:512], s0, ut[0:K, GO : GO + 511],
                            start=True, stop=False,
                        )
                        nc.tensor.matmul(
                            pt[0:128, 0:1], s0, ut[0:K, GO : GO + 1],
                            start=False, stop=False,
                        )
                        nc.tensor.matmul(
                            pt[0:128, 0:512], s1, ut[0:K, GO : GO + 512],
                            start=False, stop=False,
                        )
                        nc.tensor.matmul(
                            pt[0:128, 0:512], s2, ut[0:K, GO + 1 : GO + 513],
                            start=False, stop=True,
                        )
                        nc.tensor.matmul(
                            pt[0:128, 512:1024], s0,
                            ut[0:K, GO + 511 : GO + 1023],
                            start=True, stop=False,
                        )
                        nc.tensor.matmul(
                            pt[0:128, 512:1024], s1,
                            ut[0:K, GO + 512 : GO + 1024],
                            start=False, stop=False,
                        )
                        nc.tensor.matmul(
                            pt[0:128, 512:1023], s2,
                            ut[0:K, GO + 513 : GO + 1024],
                            start=False, stop=False,
                        )
                        nc.tensor.matmul(
                            pt[0:128, 1023:1024], s2,
                            ut[0:K, GO + W - 1 : GO + W],
                            start=False, stop=True,
                        )
                    else:
                        # 3 clean N=512 matmuls per bank; s1 (needs no edge
                        # columns) opens each group so the PE can start while
                        # the ACT edge fills land
                        for hh in range(2):
                            ob = 512 * hh
                            nc.tensor.matmul(
                                pt[0:128, ob : ob + 512],
                                s1,
                                ut[0:K, GO + ob : GO + ob + 512],
                                start=True, stop=False,
                            )
                            nc.tensor.matmul(
                                pt[0:128, ob : ob + 512],
                                s0,
                                ut[0:K, GO - 1 + ob : GO - 1 + ob + 512],
                                start=False, stop=False,
                            )
                            nc.tensor.matmul(
                                pt[0:128, ob : ob + 512],
                                s2,
                                ut[0:K, GO + 1 + ob : GO + 1 + ob + 512],
                                start=False, stop=True,
                            )
                    if conv == 1:
                        if c == NCHUNK - 1:
                            # last chunk: per-bank combine overlaps the DVE
                            # work with the remaining matmuls (shorter tail)
                            for hh in range(2):
                                sl = slice(512 * hh, 512 * hh + 512)
                                nc.vector.tensor_mul(
                                    tt[0:M, sl],
                                    vt2[0:M, vcol + 512 * hh : vcol + 512 * hh + 512],
                                    pB[0:M, sl],
                                )
                        else:
                            nc.vector.tensor_mul(
                                tt[0:M, :], vt2[0:M, vcol : vcol + W], pB[0:M, :]
                            )

                # ---- combine + store ----
                if c == NCHUNK - 1:
                    # per-half subtract + store so the tail after the final
                    # matmul is one half's worth of DVE + DMA (PSUM-direct:
                    # the pA bank lifetime no longer matters)
                    for hh in range(2):
                        sl = slice(512 * hh, 512 * hh + 512)
                        nc.vector.tensor_sub(
                            ot2[0:M, sl], pA[0:M, sl], tt[0:M, sl]
                        )
                        nc.gpsimd.dma_start(
                            o_d[rout : rout + M, sl], ot2[0:M, sl]
                        )
                else:
                    # ACT evacuates pA (it reads PSUM fast and frees the
                    # PSUM bank early so the PE never stalls on bank reuse);
                    # the DVE subtract then runs SBUF-only
                    aSB = tp.tile([128, W], bf16, tag="aSB")
                    nc.scalar.copy(aSB[0:M, :], pA[0:M, :])
                    nc.vector.tensor_sub(
                        ot2[0:M, vcol : vcol + W], aSB[0:M, :], tt[0:M, :]
                    )
                    if half == len(pr) - 1:
                        if c >= 13:
                            oeng = nc.gpsimd
                        if len(pr) == 2:
                            r0 = rout - M  # pair chunks have equal M
                            dst = o_d[r0 : r0 + 2 * M, :].rearrange(
                                "(s p) w -> p s w", s=2
                            )
                            src = ot2[0:M, :].rearrange("p (s w) -> p s w", s=2)
                            oeng.dma_start(dst, src)
                        else:
                            oeng.dma_start(
                                o_d[rout : rout + M, :], ot2[0:M, 0:W]
                            )

    nc.compile()
    return nc


def _get_program():
    if "nc" not in _cache:
        _cache["nc"] = _build_program()
    return _cache["nc"]


def _make_in_maps(u, u_vel, w1, w2, w3):
    import ml_dtypes

    bf = ml_dtypes.bfloat16
    u = np.ascontiguousarray(np.asarray(u, np.float32).reshape(B, H, W).astype(bf))
    v = np.ascontiguousarray(np.asarray(u_vel, np.float32).reshape(B, H, W).astype(bf))
    stat = _build_stationaries(w1, w2, w3).astype(bf)
    n = IMGS_PER_CORE
    return [
        {
            "u": u[i * n : (i + 1) * n].reshape(FH, W),
            "uvel": v[i * n : (i + 1) * n].reshape(FH, W),
            "stat": stat,
        }
        for i in range(NCORES)
    ]


def kernel(u, u_vel, w1, w2, w3):
    from concourse.bass_utils import run_bass_kernel_spmd

    nc = _get_program()
    in_maps = _make_in_maps(u, u_vel, w1, w2, w3)
    res = run_bass_kernel_spmd(nc, in_maps, core_ids=list(range(NCORES)))
    out = np.empty((B, 1, H, W), np.float32)
    n = IMGS_PER_CORE
    for i in range(NCORES):
        out[i * n : (i + 1) * n, 0] = (
            res.results[i]["out"].astype(np.float32).reshape(n, H, W)
        )
    return out
